# revision 28
# baseline (speedup 1.0000x reference)
"""Trainium2 Bass kernel for nn_Block_42460046688864 (dense transformer block).

Reference math (B=2, T=2048, C=2048, H=16, HD=128):
    n1  = rmsnorm(x) * norm1_w
    qkv = n1 @ attn_w.T ; q,k,v per head ; q,k = rope(q,k) ; phi = elu(.)+1
    w   = (phi_q . phi_k) * scale * tril ; w /= sum(w) ; y = w @ v
    h   = y @ proj_w.T ; x2 = x + h
    ffn = gelu(rmsnorm(x2)*norm2_w @ fc_w.T) @ mlp_proj_w.T ; out = x2 + ffn

Distribution (8 NeuronCores, one NEFF, fully data-parallel):
  - rows (b*T+t, 4096 total) sharded 512/core; every core streams the FULL
    weights from its own HBM (no activation collectives at all).
  - attention is chunked linear attention (causal tril + positive elu+1
    features == prefix-state form; scale and eps cancel to ~1e-9 rel).
    The only cross-core dependency is the causal prefix state: each core's
    segment state S_seg[h] = sum_t k_t (x) [v_t | 1] is exchanged with ONE
    small ReduceScatter. Core j writes S_seg * mask[j<s, same-seq] into
    slot s, so after the add-RS core s holds exactly the sum of its
    same-sequence predecessors' states (its causal init state). The RS is
    issued right after K/V are ready and overlaps the Q projection; the
    correction q @ S_init is fused into each chunk's PSUM accumulation.
  - V is computed directly in [token, dim] layout by using n1^T chunks as
    the stationary matmul operand (no V transposes); K additionally needs
    [token, dim] for the state outer products -> 64 small DMA transposes.

Notes:
  - norm weights are folded into attn_w / fc_w on the host (exact algebra).
  - matmul operands are bf16 (fp32 PSUM accumulation); norms, residuals and
    attention numerators/denominators stay fp32 (psum) end to end.
  - weights are pre-tiled on the host into [128 x N] DMA slabs so every
    weight DMA is one contiguous >=2KB-per-partition block.
  - SBUF pools are strict LIFO per side; long-lived attention tiles live on
    the left stack, y^T on the right stack so lifetimes nest.
  - TileContext's tail drain is patched to split its semaphore waits:
    this walrus build rejects >2 sync waits on one TPB_CTRL instruction.
"""

from contextlib import ExitStack

import numpy as np
import ml_dtypes

import concourse.bass as bass
import concourse.mybir as mybir
import concourse.tile as tile
from concourse.bass_utils import run_bass_kernel_spmd
from concourse.masks import make_identity
from bass_rust import ScopedClock

F32 = mybir.dt.float32
BF16 = mybir.dt.bfloat16
AF = mybir.ActivationFunctionType

N_CORES = 8
B, T, C, H, HD = 2, 2048, 2048, 16, 128
F = 4 * C                  # 8192 mlp hidden
R = B * T                  # 4096 flattened rows (b-major)
R_LOC = R // N_CORES       # 512 rows per core
P = 128
EPS_NORM = 1e-5
N_RT = R_LOC // P          # 4 local row tiles
N_KC = C // P              # 16 contraction tiles over C
N_CH = N_RT                # 4 local causal chunks
N_MF = F // P              # 64 mlp-hidden tiles
HD1 = HD + 1               # state cols: [v dims | 1]
SAW = H * HD1              # 2064 = all-head state cols
MLP_CC = 256               # mlp output col-chunk
N_MCH = C // MLP_CC        # 8 col chunks

_MAX_WAITS = 1  # this walrus build rejects multi-wait instructions


def _split_excess_waits(nc):
    """Move excess semaphore waits onto same-engine NoOps ahead of the op."""
    for fn in nc.m.functions:
        for bb in fn.blocks:
            insts = list(bb.instructions)
            out = []
            for ins in insts:
                si = getattr(ins, "sync_info", None)
                waits = list(si.on_wait) if si and si.on_wait else []
                sem_waits = [w for w in waits if w.sync_type == "semaphore"]
                if len(sem_waits) > _MAX_WAITS:
                    keep = [w for w in waits if w.sync_type != "semaphore"]
                    keep += sem_waits[: _MAX_WAITS - 1] if _MAX_WAITS > 1 else []
                    extra = sem_waits[_MAX_WAITS - 1:] if _MAX_WAITS > 1 else sem_waits
                    for j in range(0, len(extra), _MAX_WAITS):
                        chunk = extra[j:j + _MAX_WAITS]
                        nop = mybir.InstNoOp(
                            name=nc.get_next_instruction_name(), ins=[], outs=[]
                        )
                        nop.engine = ins.engine
                        nop.sync_info = mybir.SyncInfo(on_wait=chunk, on_update=[])
                        out.append(nop)
                    si.on_wait[:] = keep
                out.append(ins)
            if len(out) != len(insts):
                bb.instructions[:] = out


class _TC(tile.TileContext):
    """TileContext whose tail drain splits sem waits one-per-NOP."""

    def schedule_and_allocate(self):
        ret = super().schedule_and_allocate()
        _split_excess_waits(self.nc)
        return ret

    def _drain_and_barrier(self, tick_clock, wait_clock):
        probe = self.nc.sync.nop(nofuse=True, hint="drain_waits")
        wait_clock.add_sem_waits(
            probe.ins, ScopedClock({None: tick_clock.global_clock})
        )
        si = probe.ins.sync_info
        waits = list(si.on_wait) if si and si.on_wait else []
        if len(waits) > 1:
            si.on_wait[:] = waits[:1]
            for w in waits[1:]:
                extra = self.nc.sync.nop(nofuse=True, hint="drain_waits")
                extra.ins.sync_info = mybir.SyncInfo(on_wait=[w], on_update=[])
        self.nc.sync.drain()
        self.nc.all_engine_barrier()
        popped = self.nc._tile_sem_poison_stack.pop()
        assert popped is self._sem_poison
        self.nc.clear_and_free_semaphores(list(self.sems.allocated().values()))
        self.nc.all_engine_barrier()


def build_nc():
    nc = bass.Bass(target_bir_lowering=False)

    x_loc = nc.declare_dram_parameter("x_loc", [R_LOC, C], F32, isOutput=False)
    cosr = nc.declare_dram_parameter("cosr", [P, R_LOC], BF16, isOutput=False)
    sinr = nc.declare_dram_parameter("sinr", [P, R_LOC], BF16, isOutput=False)
    maskT = nc.declare_dram_parameter("maskT", [P, P], F32, isOutput=False)
    smask = nc.declare_dram_parameter("smask", [P, N_CORES], F32, isOutput=False)
    # pre-tiled weight slabs (see _prep_inputs for layouts)
    qkw = nc.declare_dram_parameter("qkw", [2 * H, P, C], BF16, isOutput=False)
    vw = nc.declare_dram_parameter("vw", [4, N_KC, P, C // 4], BF16, isOutput=False)
    pw = nc.declare_dram_parameter("pw", [8, P, 8 * 512], BF16, isOutput=False)
    fcw = nc.declare_dram_parameter("fcw", [N_MF, P, C], BF16, isOutput=False)
    mww = nc.declare_dram_parameter(
        "mww", [N_MCH, P, N_MF * MLP_CC], BF16, isOutput=False
    )
    out_loc = nc.declare_dram_parameter("out_loc", [R_LOC, C], F32, isOutput=True)

    rs_in = nc.dram_tensor("rs_in", [N_CORES, P, SAW], BF16)
    rs_out = nc.dram_tensor("rs_out", [P, SAW], BF16)

    groups = [list(range(N_CORES))]

    with _TC(nc) as tc:
        stk = ExitStack()
        const = stk.enter_context(tc.tile_pool(name="const", bufs=1))
        psum = stk.enter_context(tc.tile_pool(name="psum", bufs=1, space="PSUM"))
        def ps_t(name, tag, bufs, cols=512, dtype=F32):
            return psum.tile([P, cols], dtype, name=name, tag=tag, bufs=bufs)
        ident_f32 = const.tile([P, P], F32)
        make_identity(nc, ident_f32)
        ident_bf = const.tile([P, P], BF16)
        make_identity(nc, ident_bf)
        mask_sb = const.tile([P, P], F32)
        nc.sync.dma_start(out=mask_sb[:], in_=maskT[:, :])
        smask_sb = const.tile([P, N_CORES], F32)
        nc.sync.dma_start(out=smask_sb[:], in_=smask[:, :])
        eps_t = const.tile([P, 1], F32)
        nc.vector.memset(eps_t[:], EPS_NORM)
        cos_sb = const.tile([P, R_LOC], BF16)
        sin_sb = const.tile([P, R_LOC], BF16)
        nc.sync.dma_start(out=cos_sb[:], in_=cosr[:, :])
        nc.sync.dma_start(out=sin_sb[:], in_=sinr[:, :])

        # x2 residual at the bottom of the left stack (outlives attention);
        # phase 0 loads x straight into it (it is read again at phase 5).
        x2_ctx = ExitStack()
        x2_pool = x2_ctx.enter_context(tc.tile_pool(name="x2res", bufs=1))
        x2_res = [x2_pool.tile([P, C], F32, name=f"x2_{i}") for i in range(N_RT)]

        # attention residents (left): released together after phase 4.
        att_ctx = ExitStack()
        qk_pool = att_ctx.enter_context(tc.tile_pool(name="qkres", bufs=1))
        vp_pool = att_ctx.enter_context(tc.tile_pool(name="vpres", bufs=1))
        sbf_pool = att_ctx.enter_context(tc.tile_pool(name="sbfres", bufs=1))
        n1T_ctx = ExitStack()
        n1T_pool = n1T_ctx.enter_context(tc.tile_pool(name="n1T", bufs=1))
        n1T = [n1T_pool.tile([P, R_LOC], BF16, name=f"n1T{k}") for k in range(N_KC)]
        kres = [qk_pool.tile([P, R_LOC], BF16, name=f"k{h}") for h in range(H)]
        qres = [qk_pool.tile([P, R_LOC], BF16, name=f"q{h}") for h in range(H)]
        vp = [
            [vp_pool.tile([P, HD1], BF16, name=f"vp{h}_{i}") for i in range(N_CH)]
            for h in range(H)
        ]

        # ---- phase 0: rmsnorm(x) -> n1T (SBUF-resident, transposed) ----
        with (
            tc.tile_pool(name="p0sq", bufs=1) as p0sq,
            tc.tile_pool(name="p0st", bufs=8) as p0st,
            tc.tile_pool(name="p0n", bufs=1) as p0n,
        ):
            n_ts = []
            for i in range(N_RT):
                nc.sync.dma_start(
                    out=x2_res[i][:], in_=x_loc[i * P:(i + 1) * P, :]
                )
                sq = p0sq.tile([P, C], F32, name=f"sq{i}", tag="sq")
                ss = p0st.tile([P, 1], F32, name=f"ss{i}", tag="ss")
                nc.scalar.activation(sq[:], x2_res[i][:], AF.Square, accum_out=ss[:])
                rms = p0st.tile([P, 1], F32, name=f"rms{i}", tag="rms")
                nc.scalar.activation(
                    rms[:], ss[:], AF.Sqrt, bias=eps_t[:], scale=1.0 / C
                )
                inv = p0st.tile([P, 1], F32, name=f"inv{i}", tag="inv")
                nc.vector.reciprocal(inv[:], rms[:])
                n_t = p0n.tile([P, C], F32, name=f"n{i}", tag=f"n{i}")
                nc.vector.tensor_scalar_mul(n_t[:], x2_res[i][:], inv[:])
                n_ts.append(n_t)
            for k in range(N_KC):
                ps = ps_t(f"tr{k}", "v", 2)
                for i in range(N_RT):
                    nc.tensor.transpose(
                        ps[:, i * P:(i + 1) * P],
                        n_ts[i][:, k * P:(k + 1) * P], ident_f32[:],
                    )
                nc.scalar.copy(n1T[k][:], ps[:])

        # ---- phase 1: K then V then Q projections --------------------
        HF = HD // 2

        def rope_elu(dst, ps, pool, uid):
            """psum [128 x 512] (d-major head tile) -> phi(rope(.)) bf16."""
            raw = pool.tile([P, R_LOC], BF16, name=f"raw{uid}", tag="raw")
            nc.scalar.copy(raw[:], ps[:])
            s1 = pool.tile([HF, R_LOC], BF16, name=f"s1{uid}", tag="s1")
            s2 = pool.tile([HF, R_LOC], BF16, name=f"s2{uid}", tag="s2")
            ro = pool.tile([P, R_LOC], BF16, name=f"ro{uid}", tag="ro")
            nc.vector.tensor_mul(s1[:], raw[0:HF, :], cos_sb[0:HF, :])
            nc.vector.tensor_mul(s2[:], raw[HF:P, :], sin_sb[HF:P, :])
            nc.vector.tensor_sub(ro[0:HF, :], s1[:], s2[:])
            nc.vector.tensor_mul(s1[:], raw[0:HF, :], sin_sb[0:HF, :])
            nc.vector.tensor_mul(s2[:], raw[HF:P, :], cos_sb[HF:P, :])
            nc.vector.tensor_add(ro[HF:P, :], s1[:], s2[:])
            # phi = elu(ro)+1 = max(ro,0) + exp(min(ro,0))
            rl = pool.tile([P, R_LOC], BF16, name=f"rl{uid}", tag="rl")
            nc.vector.tensor_scalar_max(rl[:], ro[:], 0.0)
            dm = pool.tile([P, R_LOC], BF16, name=f"dm{uid}", tag="dm")
            nc.vector.tensor_scalar_min(dm[:], ro[:], 0.0)
            ex = pool.tile([P, R_LOC], BF16, name=f"ex{uid}", tag="ex")
            nc.scalar.activation(ex[:], dm[:], AF.Exp)
            nc.vector.tensor_add(dst[:], rl[:], ex[:])

        with (
            tc.tile_pool(name="p1w", bufs=4) as p1w,
            tc.tile_pool(name="p1vw", bufs=1) as p1vw,
            tc.tile_pool(name="p1r", bufs=4) as p1r,
        ):
            # K projections (j = 0..15), rope+elu on DVE as tiles land
            for j in range(H):
                w_t = p1w.tile([P, C], BF16, name=f"kw{j}", tag="qkw")
                nc.sync.dma_start(out=w_t[:], in_=qkw[j, :, :])
                ps = ps_t(f"kps{j}", "kq", 3)
                for k in range(N_KC):
                    nc.tensor.matmul(
                        ps[:], w_t[:, k * P:(k + 1) * P], n1T[k][:],
                        start=(k == 0), stop=(k == N_KC - 1),
                    )
                rope_elu(kres[j], ps, p1r, f"k{j}")

            # V projections, directly transposed: [t x hv] chunks
            if True:
                for qtr in range(4):
                    vw_sb = []
                    for k in range(N_KC):
                        w_t = p1vw.tile(
                            [P, C // 4], BF16, name=f"vw{qtr}_{k}", tag=f"vw{k}"
                        )
                        nc.sync.dma_start(out=w_t[:], in_=vw[qtr, k, :, :])
                        vw_sb.append(w_t)
                    for i in range(N_CH):
                        icol = slice(i * P, (i + 1) * P)
                        ps = ps_t(f"v{qtr}_{i}", "v", 2)
                        for k in range(N_KC):
                            nc.tensor.matmul(
                                ps[:], n1T[k][:, icol], vw_sb[k][:],
                                start=(k == 0), stop=(k == N_KC - 1),
                            )
                        for hs in range(4):
                            h = qtr * 4 + hs
                            nc.scalar.copy(
                                vp[h][i][:, 0:HD], ps[:, hs * P:(hs + 1) * P]
                            )
                            nc.vector.memset(vp[h][i][:, HD:HD1], 1.0)

            # ---- phase 2: segment states + masked RS exchange --------
            # (emitted before Q so the collective overlaps Q/scores)
            s_bf = [[None] * N_CH for _ in range(H)]
            with (
                tc.high_priority(),
                tc.tile_pool(name="p2kp", bufs=8) as p2kp,
                tc.tile_pool(name="p2all", bufs=1) as p2all,
                tc.tile_pool(name="p2msk", bufs=2) as p2msk,
            ):
                s_all = p2all.tile([P, SAW], BF16, name="s_all")
                for h in range(H):
                    kp_t = []
                    for i in range(N_CH):
                        tcol = slice(i * P, (i + 1) * P)
                        kps = ps_t(f"kptr{h}_{i}", "kptr", 1, cols=P, dtype=BF16)
                        nc.tensor.transpose(kps[:], kres[h][:, tcol], ident_bf[:])
                        kp = p2kp.tile([P, P], BF16, name=f"kp{h}_{i}", tag="kp")
                        nc.scalar.copy(kp[:], kps[:])
                        kp_t.append(kp)
                    # local prefix states P_m = sum_{i<m} kp_i^T @ [v_i | 1]
                    # (redundant accumulations: no cross-engine scan chain)
                    for m in range(1, N_CH + 1):
                        ps = ps_t(f"pfx{h}_{m}", "sd", 2, cols=HD1)
                        for i in range(m):
                            nc.tensor.matmul(
                                ps[:], kp_t[i][:], vp[h][i][:],
                                start=(i == 0), stop=(i == m - 1),
                            )
                        if m < N_CH:
                            sb = sbf_pool.tile([P, HD1], BF16, name=f"sbf{h}_{m}")
                            nc.vector.tensor_scalar_add(sb[:], ps[:], 0.0)
                            s_bf[h][m] = sb
                        else:
                            nc.vector.tensor_scalar_add(
                                s_all[:, h * HD1:(h + 1) * HD1], ps[:], 0.0
                            )
                for s in range(N_CORES):
                    ms = p2msk.tile([P, SAW], BF16, name=f"ms{s}", tag="ms")
                    nc.vector.tensor_scalar_mul(ms[:], s_all[:], smask_sb[:, s:s + 1])
                    nc.sync.dma_start(out=rs_in[s, :, :], in_=ms[:])
                nc.gpsimd.collective_compute(
                    "ReduceScatter",
                    mybir.AluOpType.add,
                    ins=[rs_in.ap().opt()],
                    outs=[rs_out.ap().opt()],
                    replica_groups=groups,
                )

            # Q projections (j = 16..31), overlap the collective
            for j in range(H):
                w_t = p1w.tile([P, C], BF16, name=f"qw{j}", tag="qkw")
                nc.sync.dma_start(out=w_t[:], in_=qkw[H + j, :, :])
                ps = ps_t(f"qps{j}", "kq", 3)
                for k in range(N_KC):
                    nc.tensor.matmul(
                        ps[:], w_t[:, k * P:(k + 1) * P], n1T[k][:],
                        start=(k == 0), stop=(k == N_KC - 1),
                    )
                rope_elu(qres[j], ps, p1r, f"q{j}")

        n1T_ctx.close()

        # right-stack pools, bottom->top in release order (LIFO per side):
        # p6w (dies after fc) | yT (dies after proj) | p5w (dies after proj)
        p6w_ctx = ExitStack()
        p6w = p6w_ctx.enter_context(tc.tile_pool(name="p6w", bufs=3, side="right"))
        yT_ctx = ExitStack()
        yT_pool = yT_ctx.enter_context(tc.tile_pool(name="yT", bufs=1, side="right"))
        yT = [
            [yT_pool.tile([P, P], BF16, name=f"yT{h}_{i}") for i in range(N_CH)]
            for h in range(H)
        ]
        pw_ctx = ExitStack()
        p5w = pw_ctx.enter_context(tc.tile_pool(name="p5w", bufs=1, side="right"))

        # ---- phase 4: scores + y = (q@S_loc + Am^T@V' + q@S_init)/den --
        with (
            tc.tile_pool(name="p4si", bufs=1) as p4si,
            tc.tile_pool(name="p4am", bufs=4) as p4am,
            tc.tile_pool(name="p4y", bufs=4) as p4y,
        ):
            sinit = p4si.tile([P, SAW], BF16, name="sinit")
            with tc.high_priority():
                nc.sync.dma_start(out=sinit[:], in_=rs_out[:, :])
            for h in range(H):
                hcol = slice(h * HD1, (h + 1) * HD1)
                for i in range(N_CH):
                    tcol = slice(i * P, (i + 1) * P)
                    a_ps = ps_t(f"a{h}_{i}", "sd", 2, cols=P)
                    nc.tensor.matmul(
                        a_ps[:], kres[h][:, tcol], qres[h][:, tcol],
                        start=True, stop=True,
                    )
                    am_t = p4am.tile([P, P], BF16, name=f"am{h}_{i}", tag="am")
                    nc.vector.tensor_mul(am_t[:], a_ps[:], mask_sb[:])
                    y_ps = ps_t(f"y{h}_{i}", "kq", 3, cols=HD1)
                    if i > 0:
                        nc.tensor.matmul(
                            y_ps[:], qres[h][:, tcol], s_bf[h][i][:],
                            start=True, stop=False,
                        )
                    nc.tensor.matmul(
                        y_ps[:], am_t[:], vp[h][i][:],
                        start=(i == 0), stop=False,
                    )
                    nc.tensor.matmul(
                        y_ps[:], qres[h][:, tcol], sinit[:, hcol],
                        start=False, stop=True,
                    )
                    rec = p4y.tile([P, 1], F32, name=f"rec{h}_{i}", tag="rec")
                    nc.vector.reciprocal(rec[:], y_ps[:, HD:HD1])
                    yb = p4y.tile([P, HD], BF16, name=f"yb{h}_{i}", tag="yb")
                    nc.vector.tensor_scalar_mul(yb[:], y_ps[:, 0:HD], rec[:])
                    tr = ps_t(f"ytr{h}_{i}", "v", 2, cols=P, dtype=BF16)
                    nc.tensor.transpose(tr[:], yb[:], ident_bf[:])
                    nc.scalar.copy(yT[h][i][:], tr[:])
        att_ctx.close()

        # ---- phase 5: proj, residual, rmsnorm2 -> n2T ----------------
        n2T_ctx = ExitStack()
        n2T_pool = n2T_ctx.enter_context(tc.tile_pool(name="n2T", bufs=1))
        n2T = [n2T_pool.tile([P, R_LOC], BF16, name=f"n2T{k}") for k in range(N_KC)]
        with (
            tc.tile_pool(name="p5sq", bufs=1) as p5sq,
            tc.tile_pool(name="p5st", bufs=8) as p5st,
            tc.tile_pool(name="p5n", bufs=2) as p5n,
        ):
            pw_sb = []
            for s in range(8):
                w_t = p5w.tile([P, 8 * 512], BF16, name=f"pw{s}")
                nc.sync.dma_start(out=w_t[:], in_=pw[s, :, :])
                pw_sb.append(w_t)
            for mt in range(N_RT):
                for ont in range(4):
                    ocol = slice(ont * 512, (ont + 1) * 512)
                    ps = ps_t(f"h{mt}_{ont}", "kq", 3)
                    for kd in range(N_KC):
                        sl = pw_sb[2 * ont + kd // 8]
                        nc.tensor.matmul(
                            ps[:], yT[kd][mt][:],
                            sl[:, (kd % 8) * 512:(kd % 8 + 1) * 512],
                            start=(kd == 0), stop=(kd == N_KC - 1),
                        )
                    nc.vector.tensor_add(
                        x2_res[mt][:, ocol], x2_res[mt][:, ocol], ps[:]
                    )
                sq = p5sq.tile([P, C], F32, name=f"sq2_{mt}", tag="sq2")
                ss = p5st.tile([P, 1], F32, name=f"ss2_{mt}", tag="ss2")
                nc.scalar.activation(sq[:], x2_res[mt][:], AF.Square, accum_out=ss[:])
                rms = p5st.tile([P, 1], F32, name=f"rms2_{mt}", tag="rms2")
                nc.scalar.activation(
                    rms[:], ss[:], AF.Sqrt, bias=eps_t[:], scale=1.0 / C
                )
                inv = p5st.tile([P, 1], F32, name=f"inv2_{mt}", tag="inv2")
                nc.vector.reciprocal(inv[:], rms[:])
                n_t = p5n.tile([P, C], F32, name=f"n2_{mt}", tag="n2")
                nc.vector.tensor_scalar_mul(n_t[:], x2_res[mt][:], inv[:])
                for k in range(N_KC):
                    tp = ps_t(f"tr2_{mt}_{k}", "v", 2)
                    nc.tensor.transpose(
                        tp[:, 0:P], n_t[:, k * P:(k + 1) * P], ident_f32[:]
                    )
                    nc.scalar.copy(n2T[k][:, mt * P:(mt + 1) * P], tp[:, 0:P])
        pw_ctx.close()
        yT_ctx.close()

        # ---- phase 6: fc + gelu -> gT (resident) ---------------------
        gT_ctx = ExitStack()
        gT_pool = gT_ctx.enter_context(tc.tile_pool(name="gT", bufs=1))
        gT = [gT_pool.tile([P, R_LOC], BF16, name=f"gT{mf}") for mf in range(N_MF)]
        p7w_ctx = ExitStack()
        p7w = p7w_ctx.enter_context(tc.tile_pool(name="p7w", bufs=2))
        with (
            tc.tile_pool(name="p6w", bufs=3) as p6w,
        ):
            for mf in range(N_MF):
                w_t = p6w.tile([P, C], BF16, name=f"fcw{mf}", tag="fcw")
                nc.sync.dma_start(out=w_t[:], in_=fcw[mf, :, :])
                ps = ps_t(f"g{mf}", "kq", 3)
                for k in range(N_KC):
                    nc.tensor.matmul(
                        ps[:], w_t[:, k * P:(k + 1) * P], n2T[k][:],
                        start=(k == 0), stop=(k == N_KC - 1),
                    )
                nc.scalar.activation(gT[mf][:], ps[:], AF.Gelu)

        # ---- phase 7: mlp proj + residual -> out ---------------------
        with (
            tc.tile_pool(name="p7o", bufs=4) as p7o,
        ):
            for ch in range(N_MCH):
                w_t = p7w.tile([P, N_MF * MLP_CC], BF16, name=f"mw{ch}", tag="mw")
                nc.sync.dma_start(out=w_t[:], in_=mww[ch, :, :])
                for mt in range(N_RT):
                    mcol = slice(mt * P, (mt + 1) * P)
                    ps = ps_t(f"f{ch}_{mt}", "kq", 3, cols=MLP_CC)
                    for kf in range(N_MF):
                        nc.tensor.matmul(
                            ps[:],
                            gT[kf][:, mcol],
                            w_t[:, kf * MLP_CC:(kf + 1) * MLP_CC],
                            start=(kf == 0), stop=(kf == N_MF - 1),
                        )
                    o_t = p7o.tile([P, MLP_CC], F32, name=f"o{ch}_{mt}", tag="o")
                    nc.vector.tensor_add(
                        o_t[:],
                        x2_res[mt][:, ch * MLP_CC:(ch + 1) * MLP_CC],
                        ps[:],
                    )
                    nc.scalar.dma_start(
                        out=out_loc[
                            mt * P:(mt + 1) * P,
                            ch * MLP_CC:(ch + 1) * MLP_CC,
                        ],
                        in_=o_t[:],
                    )
        p7w_ctx.close()
        gT_ctx.close()
        n2T_ctx.close()
        p6w_ctx.close()
        x2_ctx.close()
        stk.close()

    return nc


_NC_CACHE = None


def _get_nc():
    global _NC_CACHE
    if _NC_CACHE is None:
        _NC_CACHE = build_nc()
    return _NC_CACHE


def _prep_inputs(x, cos, sin, attention_bias, norm1_w, norm2_w, attn_w, proj_w,
                 fc_w, mlp_proj_w):
    bf = ml_dtypes.bfloat16
    xf = np.asarray(x, np.float32).reshape(R, C)
    w1 = np.asarray(norm1_w, np.float32)
    w2 = np.asarray(norm2_w, np.float32)
    aw = np.asarray(attn_w, np.float32) * w1[None, :]      # [3C, C] (norm folded)
    pwf = np.asarray(proj_w, np.float32)                   # [C, C]
    fwf = np.asarray(fc_w, np.float32) * w2[None, :]       # [F, C]
    mwf = np.asarray(mlp_proj_w, np.float32)               # [C, F]
    cosf = np.asarray(cos, np.float32)                     # [T, 64]
    sinf = np.asarray(sin, np.float32)

    awr = aw.reshape(H, 3, HD, C)
    # qkw[j<H] = K-weights of head j; qkw[j>=H] = Q-weights of head j-H.
    # qkw[j, p, k*128+m] = awr[h, comp, m, k*128+p]
    qk = np.empty((2 * H, P, C), np.float32)
    for h in range(H):
        qk[h] = awr[h, 1].T.reshape(N_KC, P, HD).transpose(1, 0, 2).reshape(P, C)
        qk[H + h] = awr[h, 0].T.reshape(N_KC, P, HD).transpose(1, 0, 2).reshape(P, C)
    # vw[half, k, p, (h-8*half)*128+d] = awr[h, 2, d, k*128+p]
    vwt = (
        awr[:, 2].reshape(H * HD, C).T.reshape(N_KC, P, 4, C // 4)
        .transpose(2, 0, 1, 3)
    )
    # pw[2*ont+half, p, (kd-8*half)*512+co] = proj_w[ont*512+co, kd*128+p]
    pwt = np.ascontiguousarray(
        pwf.reshape(4, 512, 2, 8, P).transpose(0, 2, 4, 3, 1)
    ).reshape(8, P, 8 * 512)
    # fcw[mf, p, k*128+f] = fwf[mf*128+f, k*128+p]
    fct = np.ascontiguousarray(
        fwf.reshape(N_MF, P, N_KC, P).transpose(0, 3, 2, 1)
    ).reshape(N_MF, P, C)
    # mww[ch, p, kf*CC+c] = mwf[ch*CC+c, kf*128+p]
    mwt = np.ascontiguousarray(
        mwf.reshape(N_MCH, MLP_CC, N_MF, P).transpose(0, 3, 2, 1)
    ).reshape(N_MCH, P, N_MF * MLP_CC)

    qk_b = np.ascontiguousarray(qk).astype(bf)
    vw_b = np.ascontiguousarray(vwt).astype(bf)
    pw_b = np.ascontiguousarray(pwt).astype(bf)
    fc_b = fct.astype(bf)
    mw_b = mwt.astype(bf)
    # mask[s, t] = 1 iff s <= t  (transposed causal tril)
    maskT = np.triu(np.ones((P, P), np.float32))

    in_maps = []
    for c in range(N_CORES):
        t0 = (c % (N_CORES // B)) * R_LOC
        sm = np.zeros((P, N_CORES), np.float32)
        for s in range(N_CORES):
            if s // (N_CORES // B) == c // (N_CORES // B) and s > c:
                sm[:, s] = 1.0
        in_maps.append({
            "x_loc": np.ascontiguousarray(xf[R_LOC * c:R_LOC * (c + 1)]),
            "cosr": np.ascontiguousarray(
                np.tile(cosf[t0:t0 + R_LOC].T, (2, 1))).astype(bf),
            "sinr": np.ascontiguousarray(
                np.tile(sinf[t0:t0 + R_LOC].T, (2, 1))).astype(bf),
            "maskT": maskT,
            "smask": sm,
            "qkw": qk_b,
            "vw": vw_b,
            "pw": pw_b,
            "fcw": fc_b,
            "mww": mw_b,
        })
    return in_maps


def kernel(**inputs):
    nc = _get_nc()
    in_maps = _prep_inputs(**inputs)
    res = run_bass_kernel_spmd(nc, in_maps, list(range(N_CORES)))
    out = np.concatenate(
        [np.asarray(res.results[c]["out_loc"], np.float32) for c in range(N_CORES)],
        axis=0,
    )
    return out.reshape(B, T, C)


# revision 29
# speedup vs baseline: 1.0143x; 1.0143x over previous
"""Trainium2 Bass kernel for nn_Block_42460046688864 (dense transformer block).

Reference math (B=2, T=2048, C=2048, H=16, HD=128):
    n1  = rmsnorm(x) * norm1_w
    qkv = n1 @ attn_w.T ; q,k,v per head ; q,k = rope(q,k) ; phi = elu(.)+1
    w   = (phi_q . phi_k) * scale * tril ; w /= sum(w) ; y = w @ v
    h   = y @ proj_w.T ; x2 = x + h
    ffn = gelu(rmsnorm(x2)*norm2_w @ fc_w.T) @ mlp_proj_w.T ; out = x2 + ffn

Distribution (8 NeuronCores, one NEFF, fully data-parallel):
  - rows (b*T+t, 4096 total) sharded 512/core; every core streams the FULL
    weights from its own HBM (no activation collectives at all).
  - attention is chunked linear attention (causal tril + positive elu+1
    features == prefix-state form; scale and eps cancel to ~1e-9 rel).
    The only cross-core dependency is the causal prefix state: each core's
    segment state S_seg[h] = sum_t k_t (x) [v_t | 1] is exchanged with ONE
    small ReduceScatter. Core j writes S_seg * mask[j<s, same-seq] into
    slot s, so after the add-RS core s holds exactly the sum of its
    same-sequence predecessors' states (its causal init state). The RS is
    issued right after K/V are ready and overlaps the Q projection; the
    correction q @ S_init is fused into each chunk's PSUM accumulation.
  - V is computed directly in [token, dim] layout by using n1^T chunks as
    the stationary matmul operand (no V transposes); K additionally needs
    [token, dim] for the state outer products -> 64 small DMA transposes.

Notes:
  - norm weights are folded into attn_w / fc_w on the host (exact algebra).
  - matmul operands are bf16 (fp32 PSUM accumulation); norms, residuals and
    attention numerators/denominators stay fp32 (psum) end to end.
  - weights are pre-tiled on the host into [128 x N] DMA slabs so every
    weight DMA is one contiguous >=2KB-per-partition block.
  - SBUF pools are strict LIFO per side; long-lived attention tiles live on
    the left stack, y^T on the right stack so lifetimes nest.
  - TileContext's tail drain is patched to split its semaphore waits:
    this walrus build rejects >2 sync waits on one TPB_CTRL instruction.
"""

from contextlib import ExitStack

import numpy as np
import ml_dtypes

import concourse.bass as bass
import concourse.mybir as mybir
import concourse.tile as tile
from concourse.bass_utils import run_bass_kernel_spmd
from concourse.masks import make_identity
from bass_rust import ScopedClock

F32 = mybir.dt.float32
BF16 = mybir.dt.bfloat16
AF = mybir.ActivationFunctionType

N_CORES = 8
B, T, C, H, HD = 2, 2048, 2048, 16, 128
F = 4 * C                  # 8192 mlp hidden
R = B * T                  # 4096 flattened rows (b-major)
R_LOC = R // N_CORES       # 512 rows per core
P = 128
EPS_NORM = 1e-5
N_RT = R_LOC // P          # 4 local row tiles
N_KC = C // P              # 16 contraction tiles over C
N_CH = N_RT                # 4 local causal chunks
N_MF = F // P              # 64 mlp-hidden tiles
HD1 = HD + 1               # state cols: [v dims | 1]
SAW = H * HD1              # 2064 = all-head state cols
MLP_CC = 256               # mlp output col-chunk
N_MCH = C // MLP_CC        # 8 col chunks

_MAX_WAITS = 1  # this walrus build rejects multi-wait instructions


def _split_excess_waits(nc):
    """Move excess semaphore waits onto same-engine NoOps ahead of the op."""
    for fn in nc.m.functions:
        for bb in fn.blocks:
            insts = list(bb.instructions)
            out = []
            for ins in insts:
                si = getattr(ins, "sync_info", None)
                waits = list(si.on_wait) if si and si.on_wait else []
                sem_waits = [w for w in waits if w.sync_type == "semaphore"]
                if len(sem_waits) > _MAX_WAITS:
                    keep = [w for w in waits if w.sync_type != "semaphore"]
                    keep += sem_waits[: _MAX_WAITS - 1] if _MAX_WAITS > 1 else []
                    extra = sem_waits[_MAX_WAITS - 1:] if _MAX_WAITS > 1 else sem_waits
                    for j in range(0, len(extra), _MAX_WAITS):
                        chunk = extra[j:j + _MAX_WAITS]
                        nop = mybir.InstNoOp(
                            name=nc.get_next_instruction_name(), ins=[], outs=[]
                        )
                        nop.engine = ins.engine
                        nop.sync_info = mybir.SyncInfo(on_wait=chunk, on_update=[])
                        out.append(nop)
                    si.on_wait[:] = keep
                out.append(ins)
            if len(out) != len(insts):
                bb.instructions[:] = out


class _TC(tile.TileContext):
    """TileContext whose tail drain splits sem waits one-per-NOP."""

    def schedule_and_allocate(self):
        ret = super().schedule_and_allocate()
        _split_excess_waits(self.nc)
        return ret

    def _drain_and_barrier(self, tick_clock, wait_clock):
        probe = self.nc.sync.nop(nofuse=True, hint="drain_waits")
        wait_clock.add_sem_waits(
            probe.ins, ScopedClock({None: tick_clock.global_clock})
        )
        si = probe.ins.sync_info
        waits = list(si.on_wait) if si and si.on_wait else []
        if len(waits) > 1:
            si.on_wait[:] = waits[:1]
            for w in waits[1:]:
                extra = self.nc.sync.nop(nofuse=True, hint="drain_waits")
                extra.ins.sync_info = mybir.SyncInfo(on_wait=[w], on_update=[])
        self.nc.sync.drain()
        self.nc.all_engine_barrier()
        popped = self.nc._tile_sem_poison_stack.pop()
        assert popped is self._sem_poison
        self.nc.clear_and_free_semaphores(list(self.sems.allocated().values()))
        self.nc.all_engine_barrier()


def build_nc():
    nc = bass.Bass(target_bir_lowering=False)

    x_loc = nc.declare_dram_parameter("x_loc", [R_LOC, C], F32, isOutput=False)
    cosr = nc.declare_dram_parameter("cosr", [P, R_LOC], BF16, isOutput=False)
    sinr = nc.declare_dram_parameter("sinr", [P, R_LOC], BF16, isOutput=False)
    maskT = nc.declare_dram_parameter("maskT", [P, P], F32, isOutput=False)
    smask = nc.declare_dram_parameter("smask", [P, N_CORES], F32, isOutput=False)
    # pre-tiled weight slabs (see _prep_inputs for layouts)
    qkw = nc.declare_dram_parameter("qkw", [2 * H, P, C], BF16, isOutput=False)
    vw = nc.declare_dram_parameter("vw", [4, N_KC, P, C // 4], BF16, isOutput=False)
    pw = nc.declare_dram_parameter("pw", [8, P, 8 * 512], BF16, isOutput=False)
    fcw = nc.declare_dram_parameter("fcw", [N_MF, P, C], BF16, isOutput=False)
    mww = nc.declare_dram_parameter(
        "mww", [N_MCH, P, N_MF * MLP_CC], BF16, isOutput=False
    )
    out_loc = nc.declare_dram_parameter("out_loc", [R_LOC, C], F32, isOutput=True)

    rs_in = nc.dram_tensor("rs_in", [N_CORES, P, SAW], BF16)
    rs_out = nc.dram_tensor("rs_out", [P, SAW], BF16)

    groups = [list(range(N_CORES))]

    with _TC(nc) as tc:
        stk = ExitStack()
        const = stk.enter_context(tc.tile_pool(name="const", bufs=1))
        psum = stk.enter_context(tc.tile_pool(name="psum", bufs=1, space="PSUM"))
        def ps_t(name, tag, bufs, cols=512, dtype=F32):
            return psum.tile([P, cols], dtype, name=name, tag=tag, bufs=bufs)
        ident_f32 = const.tile([P, P], F32)
        make_identity(nc, ident_f32)
        ident_bf = const.tile([P, P], BF16)
        make_identity(nc, ident_bf)
        mask_sb = const.tile([P, P], F32)
        nc.sync.dma_start(out=mask_sb[:], in_=maskT[:, :])
        smask_sb = const.tile([P, N_CORES], F32)
        nc.sync.dma_start(out=smask_sb[:], in_=smask[:, :])
        eps_t = const.tile([P, 1], F32)
        nc.vector.memset(eps_t[:], EPS_NORM)
        cos_sb = const.tile([P, R_LOC], BF16)
        sin_sb = const.tile([P, R_LOC], BF16)
        nc.sync.dma_start(out=cos_sb[:], in_=cosr[:, :])
        nc.sync.dma_start(out=sin_sb[:], in_=sinr[:, :])

        # x2 residual at the bottom of the left stack (outlives attention);
        # phase 0 loads x straight into it (it is read again at phase 5).
        x2_ctx = ExitStack()
        x2_pool = x2_ctx.enter_context(tc.tile_pool(name="x2res", bufs=1))
        x2_res = [x2_pool.tile([P, C], F32, name=f"x2_{i}") for i in range(N_RT)]

        # attention residents (left): released together after phase 4.
        att_ctx = ExitStack()
        qk_pool = att_ctx.enter_context(tc.tile_pool(name="qkres", bufs=1))
        vp_pool = att_ctx.enter_context(tc.tile_pool(name="vpres", bufs=1))
        sbf_pool = att_ctx.enter_context(tc.tile_pool(name="sbfres", bufs=1))
        n1T_ctx = ExitStack()
        n1T_pool = n1T_ctx.enter_context(tc.tile_pool(name="n1T", bufs=1))
        n1T = [n1T_pool.tile([P, R_LOC], BF16, name=f"n1T{k}") for k in range(N_KC)]
        kres = [qk_pool.tile([P, R_LOC], BF16, name=f"k{h}") for h in range(H)]
        qres = [qk_pool.tile([P, R_LOC], BF16, name=f"q{h}") for h in range(H)]
        vp = [
            [vp_pool.tile([P, HD1], BF16, name=f"vp{h}_{i}") for i in range(N_CH)]
            for h in range(H)
        ]

        # ---- phase 0: rmsnorm(x) -> n1T (SBUF-resident, transposed) ----
        with (
            tc.tile_pool(name="p0x", bufs=2) as p0x,
            tc.tile_pool(name="p0sq", bufs=1) as p0sq,
            tc.tile_pool(name="p0st", bufs=8) as p0st,
            tc.tile_pool(name="p0n", bufs=1) as p0n,
        ):
            n_ts = []
            for i in range(N_RT):
                x_t = p0x.tile([P, C], F32, name=f"x{i}", tag="x")
                nc.sync.dma_start(out=x_t[:], in_=x_loc[i * P:(i + 1) * P, :])
                sq = p0sq.tile([P, C], F32, name=f"sq{i}", tag="sq")
                ss = p0st.tile([P, 1], F32, name=f"ss{i}", tag="ss")
                nc.scalar.activation(sq[:], x_t[:], AF.Square, accum_out=ss[:])
                rms = p0st.tile([P, 1], F32, name=f"rms{i}", tag="rms")
                nc.scalar.activation(
                    rms[:], ss[:], AF.Sqrt, bias=eps_t[:], scale=1.0 / C
                )
                inv = p0st.tile([P, 1], F32, name=f"inv{i}", tag="inv")
                nc.vector.reciprocal(inv[:], rms[:])
                n_t = p0n.tile([P, C], F32, name=f"n{i}", tag=f"n{i}")
                nc.vector.tensor_scalar_mul(n_t[:], x_t[:], inv[:])
                n_ts.append(n_t)
            for k in range(N_KC):
                ps = ps_t(f"tr{k}", "v", 2)
                for i in range(N_RT):
                    nc.tensor.transpose(
                        ps[:, i * P:(i + 1) * P],
                        n_ts[i][:, k * P:(k + 1) * P], ident_f32[:],
                    )
                nc.scalar.copy(n1T[k][:], ps[:])

        # ---- phase 1: K then V then Q projections --------------------
        HF = HD // 2

        def rope_elu(dst, ps, pool, uid):
            """psum [128 x 512] (d-major head tile) -> phi(rope(.)) bf16."""
            raw = pool.tile([P, R_LOC], BF16, name=f"raw{uid}", tag="raw")
            nc.scalar.copy(raw[:], ps[:])
            s1 = pool.tile([HF, R_LOC], BF16, name=f"s1{uid}", tag="s1")
            s2 = pool.tile([HF, R_LOC], BF16, name=f"s2{uid}", tag="s2")
            ro = pool.tile([P, R_LOC], BF16, name=f"ro{uid}", tag="ro")
            nc.vector.tensor_mul(s1[:], raw[0:HF, :], cos_sb[0:HF, :])
            nc.vector.tensor_mul(s2[:], raw[HF:P, :], sin_sb[HF:P, :])
            nc.vector.tensor_sub(ro[0:HF, :], s1[:], s2[:])
            nc.vector.tensor_mul(s1[:], raw[0:HF, :], sin_sb[0:HF, :])
            nc.vector.tensor_mul(s2[:], raw[HF:P, :], cos_sb[HF:P, :])
            nc.vector.tensor_add(ro[HF:P, :], s1[:], s2[:])
            # phi = elu(ro)+1 = max(ro,0) + exp(min(ro,0))
            rl = pool.tile([P, R_LOC], BF16, name=f"rl{uid}", tag="rl")
            nc.vector.tensor_scalar_max(rl[:], ro[:], 0.0)
            dm = pool.tile([P, R_LOC], BF16, name=f"dm{uid}", tag="dm")
            nc.vector.tensor_scalar_min(dm[:], ro[:], 0.0)
            ex = pool.tile([P, R_LOC], BF16, name=f"ex{uid}", tag="ex")
            nc.scalar.activation(ex[:], dm[:], AF.Exp)
            nc.vector.tensor_add(dst[:], rl[:], ex[:])

        with (
            tc.tile_pool(name="p1w", bufs=4) as p1w,
            tc.tile_pool(name="p1vw", bufs=1) as p1vw,
            tc.tile_pool(name="p1r", bufs=4) as p1r,
        ):
            # K projections (j = 0..15), rope+elu on DVE as tiles land
            for j in range(H):
                w_t = p1w.tile([P, C], BF16, name=f"kw{j}", tag="qkw")
                nc.sync.dma_start(out=w_t[:], in_=qkw[j, :, :])
                ps = ps_t(f"kps{j}", "kq", 3)
                for k in range(N_KC):
                    nc.tensor.matmul(
                        ps[:], w_t[:, k * P:(k + 1) * P], n1T[k][:],
                        start=(k == 0), stop=(k == N_KC - 1),
                    )
                rope_elu(kres[j], ps, p1r, f"k{j}")

            # V projections, directly transposed: [t x hv] chunks
            if True:
                for qtr in range(4):
                    vw_sb = []
                    for k in range(N_KC):
                        w_t = p1vw.tile(
                            [P, C // 4], BF16, name=f"vw{qtr}_{k}", tag=f"vw{k}"
                        )
                        nc.sync.dma_start(out=w_t[:], in_=vw[qtr, k, :, :])
                        vw_sb.append(w_t)
                    for i in range(N_CH):
                        icol = slice(i * P, (i + 1) * P)
                        ps = ps_t(f"v{qtr}_{i}", "v", 2)
                        for k in range(N_KC):
                            nc.tensor.matmul(
                                ps[:], n1T[k][:, icol], vw_sb[k][:],
                                start=(k == 0), stop=(k == N_KC - 1),
                            )
                        for hs in range(4):
                            h = qtr * 4 + hs
                            nc.scalar.copy(
                                vp[h][i][:, 0:HD], ps[:, hs * P:(hs + 1) * P]
                            )
                            nc.vector.memset(vp[h][i][:, HD:HD1], 1.0)

            # ---- phase 2: segment states + masked RS exchange --------
            # (emitted before Q so the collective overlaps Q/scores)
            s_bf = [[None] * N_CH for _ in range(H)]
            with (
                tc.high_priority(),
                tc.tile_pool(name="p2kp", bufs=8) as p2kp,
                tc.tile_pool(name="p2all", bufs=1) as p2all,
                tc.tile_pool(name="p2msk", bufs=2) as p2msk,
            ):
                s_all = p2all.tile([P, SAW], BF16, name="s_all")
                for h in range(H):
                    kp_t = []
                    for i in range(N_CH):
                        tcol = slice(i * P, (i + 1) * P)
                        kps = ps_t(f"kptr{h}_{i}", "kptr", 1, cols=P, dtype=BF16)
                        nc.tensor.transpose(kps[:], kres[h][:, tcol], ident_bf[:])
                        kp = p2kp.tile([P, P], BF16, name=f"kp{h}_{i}", tag="kp")
                        nc.scalar.copy(kp[:], kps[:])
                        kp_t.append(kp)
                    # local prefix states P_m = sum_{i<m} kp_i^T @ [v_i | 1]
                    # (redundant accumulations: no cross-engine scan chain)
                    for m in range(1, N_CH + 1):
                        ps = ps_t(f"pfx{h}_{m}", "sd", 2, cols=HD1)
                        for i in range(m):
                            nc.tensor.matmul(
                                ps[:], kp_t[i][:], vp[h][i][:],
                                start=(i == 0), stop=(i == m - 1),
                            )
                        if m < N_CH:
                            sb = sbf_pool.tile([P, HD1], BF16, name=f"sbf{h}_{m}")
                            nc.vector.tensor_scalar_add(sb[:], ps[:], 0.0)
                            s_bf[h][m] = sb
                        else:
                            nc.vector.tensor_scalar_add(
                                s_all[:, h * HD1:(h + 1) * HD1], ps[:], 0.0
                            )
                for s in range(N_CORES):
                    ms = p2msk.tile([P, SAW], BF16, name=f"ms{s}", tag="ms")
                    nc.vector.tensor_scalar_mul(ms[:], s_all[:], smask_sb[:, s:s + 1])
                    nc.sync.dma_start(out=rs_in[s, :, :], in_=ms[:])
                nc.gpsimd.collective_compute(
                    "ReduceScatter",
                    mybir.AluOpType.add,
                    ins=[rs_in.ap().opt()],
                    outs=[rs_out.ap().opt()],
                    replica_groups=groups,
                )

            # Q projections (j = 16..31), overlap the collective
            for j in range(H):
                w_t = p1w.tile([P, C], BF16, name=f"qw{j}", tag="qkw")
                nc.sync.dma_start(out=w_t[:], in_=qkw[H + j, :, :])
                ps = ps_t(f"qps{j}", "kq", 3)
                for k in range(N_KC):
                    nc.tensor.matmul(
                        ps[:], w_t[:, k * P:(k + 1) * P], n1T[k][:],
                        start=(k == 0), stop=(k == N_KC - 1),
                    )
                rope_elu(qres[j], ps, p1r, f"q{j}")

        n1T_ctx.close()

        # right-stack pools, bottom->top in release order (LIFO per side):
        # p6w (dies after fc) | yT (dies after proj) | p5w (dies after proj)
        p6w_ctx = ExitStack()
        p6w = p6w_ctx.enter_context(tc.tile_pool(name="p6w", bufs=3, side="right"))
        yT_ctx = ExitStack()
        yT_pool = yT_ctx.enter_context(tc.tile_pool(name="yT", bufs=1, side="right"))
        yT = [
            [yT_pool.tile([P, P], BF16, name=f"yT{h}_{i}") for i in range(N_CH)]
            for h in range(H)
        ]
        pw_ctx = ExitStack()
        p5w = pw_ctx.enter_context(tc.tile_pool(name="p5w", bufs=1, side="right"))

        # ---- phase 4: scores + y = (q@S_loc + Am^T@V' + q@S_init)/den --
        with (
            tc.tile_pool(name="p4si", bufs=1) as p4si,
            tc.tile_pool(name="p4am", bufs=4) as p4am,
            tc.tile_pool(name="p4y", bufs=4) as p4y,
        ):
            sinit = p4si.tile([P, SAW], BF16, name="sinit")
            with tc.high_priority():
                nc.sync.dma_start(out=sinit[:], in_=rs_out[:, :])
            for h in range(H):
                hcol = slice(h * HD1, (h + 1) * HD1)
                for i in range(N_CH):
                    tcol = slice(i * P, (i + 1) * P)
                    a_ps = ps_t(f"a{h}_{i}", "sd", 2, cols=P)
                    nc.tensor.matmul(
                        a_ps[:], kres[h][:, tcol], qres[h][:, tcol],
                        start=True, stop=True,
                    )
                    am_t = p4am.tile([P, P], BF16, name=f"am{h}_{i}", tag="am")
                    nc.vector.tensor_mul(am_t[:], a_ps[:], mask_sb[:])
                    y_ps = ps_t(f"y{h}_{i}", "kq", 3, cols=HD1)
                    if i > 0:
                        nc.tensor.matmul(
                            y_ps[:], qres[h][:, tcol], s_bf[h][i][:],
                            start=True, stop=False,
                        )
                    nc.tensor.matmul(
                        y_ps[:], am_t[:], vp[h][i][:],
                        start=(i == 0), stop=False,
                    )
                    nc.tensor.matmul(
                        y_ps[:], qres[h][:, tcol], sinit[:, hcol],
                        start=False, stop=True,
                    )
                    rec = p4y.tile([P, 1], F32, name=f"rec{h}_{i}", tag="rec")
                    nc.vector.reciprocal(rec[:], y_ps[:, HD:HD1])
                    yb = p4y.tile([P, HD], BF16, name=f"yb{h}_{i}", tag="yb")
                    nc.vector.tensor_scalar_mul(yb[:], y_ps[:, 0:HD], rec[:])
                    tr = ps_t(f"ytr{h}_{i}", "v", 2, cols=P, dtype=BF16)
                    nc.tensor.transpose(tr[:], yb[:], ident_bf[:])
                    nc.scalar.copy(yT[h][i][:], tr[:])
        att_ctx.close()

        # ---- phase 5: proj, residual, rmsnorm2 -> n2T ----------------
        n2T_ctx = ExitStack()
        n2T_pool = n2T_ctx.enter_context(tc.tile_pool(name="n2T", bufs=1))
        n2T = [n2T_pool.tile([P, R_LOC], BF16, name=f"n2T{k}") for k in range(N_KC)]
        with (
            tc.tile_pool(name="p5sq", bufs=1) as p5sq,
            tc.tile_pool(name="p5st", bufs=8) as p5st,
            tc.tile_pool(name="p5n", bufs=2) as p5n,
        ):
            pw_sb = []
            for s in range(8):
                w_t = p5w.tile([P, 8 * 512], BF16, name=f"pw{s}")
                nc.sync.dma_start(out=w_t[:], in_=pw[s, :, :])
                pw_sb.append(w_t)
            for mt in range(N_RT):
                nc.sync.dma_start(
                    out=x2_res[mt][:], in_=x_loc[mt * P:(mt + 1) * P, :]
                )
                for ont in range(4):
                    ocol = slice(ont * 512, (ont + 1) * 512)
                    ps = ps_t(f"h{mt}_{ont}", "kq", 3)
                    for kd in range(N_KC):
                        sl = pw_sb[2 * ont + kd // 8]
                        nc.tensor.matmul(
                            ps[:], yT[kd][mt][:],
                            sl[:, (kd % 8) * 512:(kd % 8 + 1) * 512],
                            start=(kd == 0), stop=(kd == N_KC - 1),
                        )
                    nc.vector.tensor_add(
                        x2_res[mt][:, ocol], x2_res[mt][:, ocol], ps[:]
                    )
                sq = p5sq.tile([P, C], F32, name=f"sq2_{mt}", tag="sq2")
                ss = p5st.tile([P, 1], F32, name=f"ss2_{mt}", tag="ss2")
                nc.scalar.activation(sq[:], x2_res[mt][:], AF.Square, accum_out=ss[:])
                rms = p5st.tile([P, 1], F32, name=f"rms2_{mt}", tag="rms2")
                nc.scalar.activation(
                    rms[:], ss[:], AF.Sqrt, bias=eps_t[:], scale=1.0 / C
                )
                inv = p5st.tile([P, 1], F32, name=f"inv2_{mt}", tag="inv2")
                nc.vector.reciprocal(inv[:], rms[:])
                n_t = p5n.tile([P, C], F32, name=f"n2_{mt}", tag="n2")
                nc.vector.tensor_scalar_mul(n_t[:], x2_res[mt][:], inv[:])
                for k in range(N_KC):
                    tp = ps_t(f"tr2_{mt}_{k}", "v", 2)
                    nc.tensor.transpose(
                        tp[:, 0:P], n_t[:, k * P:(k + 1) * P], ident_f32[:]
                    )
                    nc.scalar.copy(n2T[k][:, mt * P:(mt + 1) * P], tp[:, 0:P])
        pw_ctx.close()
        yT_ctx.close()

        # ---- phase 6: fc + gelu -> gT (resident) ---------------------
        gT_ctx = ExitStack()
        gT_pool = gT_ctx.enter_context(tc.tile_pool(name="gT", bufs=1))
        gT = [gT_pool.tile([P, R_LOC], BF16, name=f"gT{mf}") for mf in range(N_MF)]
        p7w_ctx = ExitStack()
        p7w = p7w_ctx.enter_context(tc.tile_pool(name="p7w", bufs=2))
        with (
            tc.tile_pool(name="p6w", bufs=3) as p6w,
        ):
            for mf in range(N_MF):
                w_t = p6w.tile([P, C], BF16, name=f"fcw{mf}", tag="fcw")
                nc.sync.dma_start(out=w_t[:], in_=fcw[mf, :, :])
                ps = ps_t(f"g{mf}", "kq", 3)
                for k in range(N_KC):
                    nc.tensor.matmul(
                        ps[:], w_t[:, k * P:(k + 1) * P], n2T[k][:],
                        start=(k == 0), stop=(k == N_KC - 1),
                    )
                nc.scalar.activation(gT[mf][:], ps[:], AF.Gelu)

        # ---- phase 7: mlp proj + residual -> out ---------------------
        with (
            tc.tile_pool(name="p7o", bufs=4) as p7o,
        ):
            for ch in range(N_MCH):
                w_t = p7w.tile([P, N_MF * MLP_CC], BF16, name=f"mw{ch}", tag="mw")
                nc.sync.dma_start(out=w_t[:], in_=mww[ch, :, :])
                for mt in range(N_RT):
                    mcol = slice(mt * P, (mt + 1) * P)
                    ps = ps_t(f"f{ch}_{mt}", "kq", 3, cols=MLP_CC)
                    for kf in range(N_MF):
                        nc.tensor.matmul(
                            ps[:],
                            gT[kf][:, mcol],
                            w_t[:, kf * MLP_CC:(kf + 1) * MLP_CC],
                            start=(kf == 0), stop=(kf == N_MF - 1),
                        )
                    o_t = p7o.tile([P, MLP_CC], F32, name=f"o{ch}_{mt}", tag="o")
                    nc.vector.tensor_add(
                        o_t[:],
                        x2_res[mt][:, ch * MLP_CC:(ch + 1) * MLP_CC],
                        ps[:],
                    )
                    nc.scalar.dma_start(
                        out=out_loc[
                            mt * P:(mt + 1) * P,
                            ch * MLP_CC:(ch + 1) * MLP_CC,
                        ],
                        in_=o_t[:],
                    )
        p7w_ctx.close()
        gT_ctx.close()
        n2T_ctx.close()
        p6w_ctx.close()
        x2_ctx.close()
        stk.close()

    return nc


_NC_CACHE = None


def _get_nc():
    global _NC_CACHE
    if _NC_CACHE is None:
        _NC_CACHE = build_nc()
    return _NC_CACHE


def _prep_inputs(x, cos, sin, attention_bias, norm1_w, norm2_w, attn_w, proj_w,
                 fc_w, mlp_proj_w):
    bf = ml_dtypes.bfloat16
    xf = np.asarray(x, np.float32).reshape(R, C)
    w1 = np.asarray(norm1_w, np.float32)
    w2 = np.asarray(norm2_w, np.float32)
    aw = np.asarray(attn_w, np.float32) * w1[None, :]      # [3C, C] (norm folded)
    pwf = np.asarray(proj_w, np.float32)                   # [C, C]
    fwf = np.asarray(fc_w, np.float32) * w2[None, :]       # [F, C]
    mwf = np.asarray(mlp_proj_w, np.float32)               # [C, F]
    cosf = np.asarray(cos, np.float32)                     # [T, 64]
    sinf = np.asarray(sin, np.float32)

    awr = aw.reshape(H, 3, HD, C)
    # qkw[j<H] = K-weights of head j; qkw[j>=H] = Q-weights of head j-H.
    # qkw[j, p, k*128+m] = awr[h, comp, m, k*128+p]
    qk = np.empty((2 * H, P, C), np.float32)
    for h in range(H):
        qk[h] = awr[h, 1].T.reshape(N_KC, P, HD).transpose(1, 0, 2).reshape(P, C)
        qk[H + h] = awr[h, 0].T.reshape(N_KC, P, HD).transpose(1, 0, 2).reshape(P, C)
    # vw[half, k, p, (h-8*half)*128+d] = awr[h, 2, d, k*128+p]
    vwt = (
        awr[:, 2].reshape(H * HD, C).T.reshape(N_KC, P, 4, C // 4)
        .transpose(2, 0, 1, 3)
    )
    # pw[2*ont+half, p, (kd-8*half)*512+co] = proj_w[ont*512+co, kd*128+p]
    pwt = np.ascontiguousarray(
        pwf.reshape(4, 512, 2, 8, P).transpose(0, 2, 4, 3, 1)
    ).reshape(8, P, 8 * 512)
    # fcw[mf, p, k*128+f] = fwf[mf*128+f, k*128+p]
    fct = np.ascontiguousarray(
        fwf.reshape(N_MF, P, N_KC, P).transpose(0, 3, 2, 1)
    ).reshape(N_MF, P, C)
    # mww[ch, p, kf*CC+c] = mwf[ch*CC+c, kf*128+p]
    mwt = np.ascontiguousarray(
        mwf.reshape(N_MCH, MLP_CC, N_MF, P).transpose(0, 3, 2, 1)
    ).reshape(N_MCH, P, N_MF * MLP_CC)

    qk_b = np.ascontiguousarray(qk).astype(bf)
    vw_b = np.ascontiguousarray(vwt).astype(bf)
    pw_b = np.ascontiguousarray(pwt).astype(bf)
    fc_b = fct.astype(bf)
    mw_b = mwt.astype(bf)
    # mask[s, t] = 1 iff s <= t  (transposed causal tril)
    maskT = np.triu(np.ones((P, P), np.float32))

    in_maps = []
    for c in range(N_CORES):
        t0 = (c % (N_CORES // B)) * R_LOC
        sm = np.zeros((P, N_CORES), np.float32)
        for s in range(N_CORES):
            if s // (N_CORES // B) == c // (N_CORES // B) and s > c:
                sm[:, s] = 1.0
        in_maps.append({
            "x_loc": np.ascontiguousarray(xf[R_LOC * c:R_LOC * (c + 1)]),
            "cosr": np.ascontiguousarray(
                np.tile(cosf[t0:t0 + R_LOC].T, (2, 1))).astype(bf),
            "sinr": np.ascontiguousarray(
                np.tile(sinf[t0:t0 + R_LOC].T, (2, 1))).astype(bf),
            "maskT": maskT,
            "smask": sm,
            "qkw": qk_b,
            "vw": vw_b,
            "pw": pw_b,
            "fcw": fc_b,
            "mww": mw_b,
        })
    return in_maps


def kernel(**inputs):
    nc = _get_nc()
    in_maps = _prep_inputs(**inputs)
    res = run_bass_kernel_spmd(nc, in_maps, list(range(N_CORES)))
    out = np.concatenate(
        [np.asarray(res.results[c]["out_loc"], np.float32) for c in range(N_CORES)],
        axis=0,
    )
    return out.reshape(B, T, C)
